# revision 31
# baseline (speedup 1.0000x reference)
"""Trainium2 Bass kernel for nn_AdaptiveFourierTransformGateLayer.

Pipeline (per core, data-parallel over batch across 8 cores):
  x [256,2048,16] --DVE--> h [256,2048] --PE transpose--> hT
  hT --PE DFT matmuls (f32r)--> xr,xi [256,1024] --transpose--> xrT,xiT
  xrT --PE complex MLP layer1 (f32r)--> o1 --transpose--> o1T
  o1T --PE layer2 (split psums A,B,C,D)--> o2r,o2i -> amp --transpose--> ampT
  ampT --PE gate matmul--> logits,z -> noisy-top3 softmax -> gates [256,88]

All heavy matmuls use float32r (11-bit mantissa fp32, full PE rate at
free dim >= 256); weights pre-rounded host-side so DMA feeds f32r tiles
directly. Biases are added via k=1 ones-row matmuls into PSUM.
"""
import sys
import types
import contextlib
import ctypes

import numpy as np

if "/opt/trn_rl_repo" not in sys.path:
    sys.path.insert(0, "/opt/trn_rl_repo")

# ---------------------------------------------------------------------------
# NTFF trace hook shim (only used when trace=True; harmless otherwise)
# ---------------------------------------------------------------------------


def _install_trace_shim():
    if "antenv.axon_hooks" in sys.modules:
        return
    so_path = "/opt/axon/libaxon_pjrt.so"

    def _mk():
        try:
            lib = ctypes.CDLL(so_path)
        except OSError:
            return None
        if not hasattr(lib, "axon_start_nrt_profile"):
            return None
        lib.axon_start_nrt_profile.argtypes = [
            ctypes.POINTER(ctypes.c_int64),
            ctypes.c_size_t,
        ]
        lib.axon_start_nrt_profile.restype = ctypes.c_int64
        lib.axon_stop_nrt_profile.argtypes = [ctypes.c_char_p]
        lib.axon_stop_nrt_profile.restype = ctypes.c_int64

        @contextlib.contextmanager
        def _hook(output_dir, device_ids):
            import jax

            jax.devices()
            if device_ids:
                ids = (ctypes.c_int64 * len(device_ids))(*device_ids)
                rc = lib.axon_start_nrt_profile(ids, len(device_ids))
            else:
                rc = lib.axon_start_nrt_profile(None, 0)
            if rc != 0:
                raise RuntimeError(f"axon_start_nrt_profile rc={rc}")
            try:
                yield
            finally:
                n = lib.axon_stop_nrt_profile(str(output_dir).encode())
                print(f"profile: {n} file(s) written to {output_dir}", file=sys.stderr)

        return _hook

    mod = types.ModuleType("antenv.axon_hooks")
    mod._hook = _mk()
    mod.get_axon_ntff_profile_hook = lambda: mod._hook
    mod.set_axon_ntff_profile_hook = lambda h: setattr(mod, "_hook", h)
    sys.modules["antenv.axon_hooks"] = mod
    try:
        import antenv

        antenv.axon_hooks = mod
    except ImportError:
        pass


_install_trace_shim()

import concourse.tile as tile  # noqa: E402
from concourse import bacc, mybir  # noqa: E402
from concourse.bass_utils import run_bass_kernel_spmd  # noqa: E402
from concourse.masks import make_identity  # noqa: E402

# ---------------------------------------------------------------------------
# Problem constants (hardcoded)
# ---------------------------------------------------------------------------
B = 2048
L = 2048
CH = 16
F = 1024  # num freqs (rfft bins 1..1024)
FH = 4096  # hidden
E = 88  # num experts
TOPK = 3
NOISE_EPS = 0.01
NCORES = 8
BL = B // NCORES  # 256 rows per core
F32R = mybir.dt.float32r
F32 = mybir.dt.float32


def rnd11(x):
    """Round-to-nearest keeping 11 mantissa bits (hardware f32r rounding)."""
    a = np.ascontiguousarray(x, np.float32)
    ai = a.view(np.uint32)
    return ((ai + np.uint32(1 << 11)) & np.uint32(0xFFFFF000)).view(np.float32)


# bias_all layout offsets (elements)
OFF_B1R = 0
OFF_B1I = 4096
OFF_B2R = 8192
OFF_B2I = 9216
OFF_CSC = 10240
OFF_CSS = 11264
OFF_ONES = 12288
OFF_FCB = 12416
BIAS_LEN = 12544


def _build_program(training: bool):
    nc = bacc.Bacc("TRN2", target_bir_lowering=False, debug=False, num_devices=NCORES)

    x_d = nc.dram_tensor("x", [BL, L, CH], F32, kind="ExternalInput").ap()
    c_d = nc.dram_tensor("cdft", [L, F], F32R, kind="ExternalInput").ap()
    s_d = nc.dram_tensor("sdft", [L, F], F32R, kind="ExternalInput").ap()
    w1r_d = nc.dram_tensor("w1r", [F, FH], F32R, kind="ExternalInput").ap()
    w1i_d = nc.dram_tensor("w1i", [F, FH], F32R, kind="ExternalInput").ap()
    w2r_d = nc.dram_tensor("w2r", [FH, F], F32R, kind="ExternalInput").ap()
    w2i_d = nc.dram_tensor("w2i", [FH, F], F32R, kind="ExternalInput").ap()
    wgn_d = nc.dram_tensor("wgn", [F, 512], F32R, kind="ExternalInput").ap()
    bias_d = nc.dram_tensor("bias_all", [1, BIAS_LEN], F32R, kind="ExternalInput").ap()
    fcw_d = nc.dram_tensor("fcw", [128, CH], F32, kind="ExternalInput").ap()
    eps_d = nc.dram_tensor("eps", [128, 2, E], F32, kind="ExternalInput").ap()
    out_d = nc.dram_tensor("out", [BL, E], F32, kind="ExternalOutput").ap()

    with tile.TileContext(nc) as tc:
        with tc.tile_pool(name="acts", bufs=1) as acts, \
             tc.tile_pool(name="stream", bufs=4) as stream, \
             tc.tile_pool(name="stage", bufs=2) as stage, \
             tc.tile_pool(name="stageb", bufs=2) as stageb, \
             tc.tile_pool(name="consts", bufs=1) as consts, \
             tc.tile_pool(name="ps", bufs=2, space="PSUM") as ps:

            ident = consts.tile([128, 128], F32, tag="ident")
            make_identity(nc, ident)
            ident_r = consts.tile([128, 128], F32R, tag="identr")
            nc.vector.tensor_copy(ident_r, ident)
            fcw = consts.tile([128, CH], F32, tag="fcw")
            nc.sync.dma_start(fcw, fcw_d)
            eps_sb = consts.tile([128, 2, E], F32, tag="eps")
            nc.sync.dma_start(eps_sb, eps_d)
            ones_row = consts.tile([1, 128], F32R, tag="ones")
            nc.sync.dma_start(ones_row, bias_d[:, OFF_ONES : OFF_ONES + 128])
            fcb_row = consts.tile([1, 128], F32R, tag="fcb")
            nc.sync.dma_start(fcb_row, bias_d[:, OFF_FCB : OFF_FCB + 128])

            # persistent activation tiles
            hT = acts.tile([128, 16, 256], F32R, tag="hT")  # [l-part, lt, b]
            xrT = acts.tile([128, 8, 256], F32R, tag="xT")  # [f-part, ft, b]
            xiT = acts.tile([128, 8, 256], F32R, tag="xT2")
            o1rT = acts.tile([128, 32, 256], F32R, tag="o1rT")  # [h-part, ht, b]
            o1iT = acts.tile([128, 32, 256], F32R, tag="o1iT")

            # ---------------- Stage A: FC + transpose to hT ----------------
            scopeA = nc.named_scope("stageA_fc"); scopeA.__enter__()
            LC = 256  # l-elems per FC chunk
            fcw_t = consts.tile([128, 32, 8], F32, tag="fcwt")
            nc.vector.tensor_copy(
                fcw_t, fcw[:, 0:8].unsqueeze(1).to_broadcast([128, 32, 8]))
            fcw_b3 = fcw_t.unsqueeze(1).to_broadcast([128, LC // 32, 32, 8])
            x_v = x_d.rearrange("(bt p) l c -> bt p (l c)", bt=2)
            cv = c_d.rearrange("(g p) f -> p g f", p=128)
            sv = s_d.rearrange("(g p) f -> p g f", p=128)
            for bt in range(2):
                for lc in range(L // LC):
                    xa = stream.tile([128, LC, CH], F32, tag="stream")
                    nc.sync.dma_start(
                        xa.rearrange("p l c -> p (l c)"),
                        x_v[bt][:, lc * LC * CH : (lc + 1) * LC * CH],
                    )
                    # in-place multiply by fc_w: DVE does channels 0..7,
                    # ScalarE does 8..15 (fc_w replicated across partitions)
                    xa4 = xa.rearrange("p (g q) c -> p g q c", q=32)
                    nc.vector.tensor_tensor(
                        xa4[:, :, :, 0:8], xa4[:, :, :, 0:8], fcw_b3,
                        op=mybir.AluOpType.mult)
                    for c in range(8, 16):
                        nc.scalar.mul(xa[:, :, c], xa[:, :, c], fcw[:, c : c + 1])
                    hch = stage.tile([128, LC], F32, tag="stage")
                    nc.vector.tensor_reduce(
                        out=hch, in_=xa, op=mybir.AluOpType.add,
                        axis=mybir.AxisListType.X,
                    )
                    # transpose 128-blocks into hT
                    for j in range(LC // 128):
                        lt = (lc * LC) // 128 + j
                        pt = ps.tile([128, 128], F32, tag="pb", bufs=4)
                        nc.tensor.transpose(pt, hch[:, j * 128 : (j + 1) * 128], ident)
                        nc.vector.tensor_copy(hT[:, lt, bt * 128 : (bt + 1) * 128], pt)

            scopeA.__exit__(None, None, None)
            scopeB = nc.named_scope("stageB_dft"); scopeB.__enter__()
            for f5 in range(2):
                ps_xr = [None, None]
                ps_xi = [None, None]
                for bt in range(2):
                    ps_xr[bt] = ps.tile([128, 512], F32, tag="pa", bufs=4,
                                        name=f"ps_xr_{f5}_{bt}")
                    ps_xi[bt] = ps.tile([128, 512], F32, tag="pb", bufs=4,
                                        name=f"ps_xi_{f5}_{bt}")
                for ltg in range(2):  # groups of 8 lt
                    c_sb = stream.tile([128, 8, 512], F32R, tag="stream",
                                       name=f"c_sb_{f5}_{ltg}")
                    s_sb = stream.tile([128, 8, 512], F32R, tag="stream",
                                       name=f"s_sb_{f5}_{ltg}")
                    nc.sync.dma_start(
                        c_sb, cv[:, ltg * 8 : (ltg + 1) * 8, f5 * 512 : (f5 + 1) * 512])
                    nc.sync.dma_start(
                        s_sb, sv[:, ltg * 8 : (ltg + 1) * 8, f5 * 512 : (f5 + 1) * 512])
                    for j in range(8):
                        lt = ltg * 8 + j
                        for bt in range(2):
                            first = lt == 0
                            nc.tensor.matmul(
                                ps_xr[bt], hT[:, lt, bt * 128 : (bt + 1) * 128],
                                c_sb[:, j], start=first, stop=False)
                            nc.tensor.matmul(
                                ps_xi[bt], hT[:, lt, bt * 128 : (bt + 1) * 128],
                                s_sb[:, j], start=first, stop=False)
                # fc_b contribution: += fc_b * colsum(C/S)
                csc = stageb.tile([1, 512], F32R, tag="stageb",
                                  name=f"csc_{f5}")
                nc.sync.dma_start(
                    csc, bias_d[:, OFF_CSC + f5 * 512 : OFF_CSC + (f5 + 1) * 512])
                css = stageb.tile([1, 512], F32R, tag="stageb",
                                  name=f"css_{f5}")
                nc.sync.dma_start(
                    css, bias_d[:, OFF_CSS + f5 * 512 : OFF_CSS + (f5 + 1) * 512])
                for bt in range(2):
                    nc.tensor.matmul(ps_xr[bt], fcb_row, csc, start=False, stop=True)
                    nc.tensor.matmul(ps_xi[bt], fcb_row, css, start=False, stop=True)
                # evacuate + transpose into xrT/xiT/xiTn
                for bt in range(2):
                    st_r = stage.tile([128, 512], F32, tag="stage",
                                      name=f"dft_er_{f5}_{bt}")
                    nc.scalar.copy(st_r, ps_xr[bt])
                    st_i = stage.tile([128, 512], F32, tag="stage",
                                      name=f"dft_ei_{f5}_{bt}")
                    nc.scalar.copy(st_i, ps_xi[bt])
                    for j in range(4):
                        ft = f5 * 4 + j
                        pt = ps.tile([128, 128], F32, tag="pb", bufs=4)
                        nc.tensor.transpose(pt, st_r[:, j * 128 : (j + 1) * 128], ident)
                        nc.vector.tensor_copy(
                            xrT[:, ft, bt * 128 : (bt + 1) * 128], pt)
                        pt2 = ps.tile([128, 128], F32, tag="pb", bufs=4)
                        nc.tensor.transpose(pt2, st_i[:, j * 128 : (j + 1) * 128], ident)
                        nc.vector.tensor_copy(
                            xiT[:, ft, bt * 128 : (bt + 1) * 128], pt2)

            scopeB.__exit__(None, None, None)
            scopeC = nc.named_scope("stageC_l1"); scopeC.__enter__()
            # o1r = relu(xr@W1r - xi@W1i + b1r) ; o1i = relu(xi@W1r + xr@W1i + b1i)
            w1r_v = w1r_d.rearrange("(ft p) h -> p ft h", p=128)
            w1i_v = w1i_d.rearrange("(ft p) h -> p ft h", p=128)
            for h5 in range(8):  # h chunks of 512
                w1r_sb = stream.tile([128, 8, 512], F32R, tag="stream")
                nc.sync.dma_start(w1r_sb, w1r_v[:, :, h5 * 512 : (h5 + 1) * 512])
                w1i_sb = stream.tile([128, 8, 512], F32R, tag="stream")
                nc.sync.dma_start(w1i_sb, w1i_v[:, :, h5 * 512 : (h5 + 1) * 512])
                b1r_sl = stageb.tile([1, 512], F32R, tag="stageb")
                nc.sync.dma_start(
                    b1r_sl, bias_d[:, OFF_B1R + h5 * 512 : OFF_B1R + (h5 + 1) * 512])
                b1i_sl = stageb.tile([1, 512], F32R, tag="stageb")
                nc.sync.dma_start(
                    b1i_sl, bias_d[:, OFF_B1I + h5 * 512 : OFF_B1I + (h5 + 1) * 512])
                for bt in range(2):
                    bs = slice(bt * 128, (bt + 1) * 128)
                    # split accumulators (2 LDW : 4 MM):
                    # o1r = relu(A - B + b1r), o1i = relu(C + D + b1i)
                    pRA = ps.tile([128, 512], F32, tag="pa", bufs=4)
                    pRB = ps.tile([128, 512], F32, tag="pa", bufs=4)
                    pIC = ps.tile([128, 512], F32, tag="pb", bufs=4)
                    pID = ps.tile([128, 512], F32, tag="pb", bufs=4)
                    for ft in range(8):
                        first = ft == 0
                        nc.tensor.matmul(pRA, xrT[:, ft, bs], w1r_sb[:, ft],
                                         start=first, stop=False)
                        nc.tensor.matmul(pID, xrT[:, ft, bs], w1i_sb[:, ft],
                                         start=first, stop=(ft == 7))
                        nc.tensor.matmul(pIC, xiT[:, ft, bs], w1r_sb[:, ft],
                                         start=first, stop=False)
                        nc.tensor.matmul(pRB, xiT[:, ft, bs], w1i_sb[:, ft],
                                         start=first, stop=(ft == 7))
                    nc.tensor.matmul(pRA, ones_row, b1r_sl, start=False, stop=True)
                    nc.tensor.matmul(pIC, ones_row, b1i_sl, start=False, stop=True)
                    # evac: ACT copy, DVE combine, DVE relu-cast to f32r
                    st_r = stage.tile([128, 512], F32, tag="stage")
                    nc.scalar.copy(st_r, pRA)
                    nc.vector.tensor_sub(st_r, st_r, pRB)
                    st_i = stage.tile([128, 512], F32, tag="stage")
                    nc.scalar.copy(st_i, pIC)
                    nc.vector.tensor_add(st_i, st_i, pID)
                    o_r = stage.tile([128, 512], F32R, tag="stager", bufs=2)
                    nc.vector.tensor_scalar(o_r, st_r, 0.0, None,
                                            op0=mybir.AluOpType.max)
                    o_i = stage.tile([128, 512], F32R, tag="stager", bufs=2)
                    nc.vector.tensor_scalar(o_i, st_i, 0.0, None,
                                            op0=mybir.AluOpType.max)
                    for j in range(4):
                        ht = h5 * 4 + j
                        pt = ps.tile([128, 128], F32R, tag="pa", bufs=4)
                        nc.tensor.transpose(pt, o_r[:, j * 128 : (j + 1) * 128],
                                            ident_r)
                        nc.vector.tensor_copy(o1rT[:, ht, bs], pt)
                        pt2 = ps.tile([128, 128], F32R, tag="pb", bufs=4)
                        nc.tensor.transpose(pt2, o_i[:, j * 128 : (j + 1) * 128],
                                            ident_r)
                        nc.vector.tensor_copy(o1iT[:, ht, bs], pt2)

            scopeC.__exit__(None, None, None)
            scopeD = nc.named_scope("stageD_l2"); scopeD.__enter__()
            # o2r = (A - B) + b2r ; o2i = (C + D) + b2i
            # A=o1r@W2r B=o1i@W2i C=o1i@W2r D=o1r@W2i ; amp=sqrt(o2r^2+o2i^2)
            ampT = acts.tile([128, 8, 256], F32R, tag="ampT")
            w2r_v = w2r_d.rearrange("(hg p) f -> p hg f", p=128)
            w2i_v = w2i_d.rearrange("(hg p) f -> p hg f", p=128)
            amp_sb = consts.tile([128, 2, 2, 512], F32R, tag="ampsb")  # [p, bt, f5, f]
            for f5 in range(2):
                fs = slice(f5 * 512, (f5 + 1) * 512)
                b2r_sl = stageb.tile([1, 512], F32R, tag="stageb")
                nc.sync.dma_start(
                    b2r_sl, bias_d[:, OFF_B2R + f5 * 512 : OFF_B2R + (f5 + 1) * 512])
                b2i_sl = stageb.tile([1, 512], F32R, tag="stageb")
                nc.sync.dma_start(
                    b2i_sl, bias_d[:, OFF_B2I + f5 * 512 : OFF_B2I + (f5 + 1) * 512])
                pA = [None, None]
                pB = [None, None]
                pC = [None, None]
                pD = [None, None]
                for bt in range(2):
                    pA[bt] = ps.tile([128, 512], F32, tag="pa", bufs=4, name=f"pA_{f5}_{bt}")
                    pB[bt] = ps.tile([128, 512], F32, tag="pa", bufs=4, name=f"pB_{f5}_{bt}")
                    pC[bt] = ps.tile([128, 512], F32, tag="pb", bufs=4, name=f"pC_{f5}_{bt}")
                    pD[bt] = ps.tile([128, 512], F32, tag="pb", bufs=4, name=f"pD_{f5}_{bt}")
                for hg in range(4):  # ht groups of 8; W2 chunks loaded ONCE
                    w2r_sb = stream.tile([128, 8, 512], F32R, tag="stream")
                    nc.sync.dma_start(w2r_sb, w2r_v[:, hg * 8 : (hg + 1) * 8, fs])
                    w2i_sb = stream.tile([128, 8, 512], F32R, tag="stream")
                    nc.sync.dma_start(w2i_sb, w2i_v[:, hg * 8 : (hg + 1) * 8, fs])
                    for j in range(8):
                        ht = hg * 8 + j
                        first = ht == 0
                        for bt in range(2):
                            bs = slice(bt * 128, (bt + 1) * 128)
                            # o1rT stationary reused for A and D; o1iT for C, B
                            nc.tensor.matmul(pA[bt], o1rT[:, ht, bs], w2r_sb[:, j],
                                             start=first, stop=False)
                            nc.tensor.matmul(pD[bt], o1rT[:, ht, bs], w2i_sb[:, j],
                                             start=first, stop=(ht == 31))
                            nc.tensor.matmul(pC[bt], o1iT[:, ht, bs], w2r_sb[:, j],
                                             start=first, stop=False)
                            nc.tensor.matmul(pB[bt], o1iT[:, ht, bs], w2i_sb[:, j],
                                             start=first, stop=(ht == 31))
                for bt in range(2):
                    nc.tensor.matmul(pA[bt], ones_row, b2r_sl, start=False, stop=True)
                    nc.tensor.matmul(pC[bt], ones_row, b2i_sl, start=False, stop=True)
                    # amp = sqrt((A-B)^2 + (C+D)^2) -> amp_sb (transposed later)
                    t_r = stage.tile([128, 512], F32, tag="stage")
                    nc.scalar.copy(t_r, pA[bt])
                    nc.vector.tensor_sub(t_r, t_r, pB[bt])
                    t_i = stage.tile([128, 512], F32, tag="stage")
                    nc.scalar.copy(t_i, pC[bt])
                    nc.vector.tensor_add(t_i, t_i, pD[bt])
                    nc.scalar.square(t_r, t_r)
                    nc.scalar.square(t_i, t_i)
                    nc.vector.tensor_add(t_r, t_r, t_i)
                    nc.scalar.sqrt(amp_sb[:, bt, f5], t_r)
            # deferred transposes amp[b, f] -> ampT[f-part, ft, b]
            for bt in range(2):
                bs = slice(bt * 128, (bt + 1) * 128)
                for f5 in range(2):
                    for j in range(4):
                        ft = f5 * 4 + j
                        pt = ps.tile([128, 128], F32R, tag="pa", bufs=4)
                        nc.tensor.transpose(
                            pt, amp_sb[:, bt, f5, j * 128 : (j + 1) * 128], ident_r)
                        nc.vector.tensor_copy(ampT[:, ft, bs], pt)
            scopeD.__exit__(None, None, None)
            scopeE = nc.named_scope("stageE_gate"); scopeE.__enter__()
            wgn_sb = stream.tile([128, 8, 512], F32R, tag="stream")
            nc.sync.dma_start(wgn_sb, wgn_d.rearrange("(ft p) n -> p ft n", p=128))
            out_v = out_d.rearrange("(bt p) e -> bt p e", bt=2)
            for bt in range(2):
                bs = slice(bt * 128, (bt + 1) * 128)
                pg = ps.tile([128, 512], F32, tag="pa", bufs=4)
                for ft in range(8):
                    nc.tensor.matmul(pg, ampT[:, ft, bs], wgn_sb[:, ft],
                                     start=(ft == 0), stop=(ft == 7))
                logits = stage.tile([128, E], F32, tag="logits", bufs=2)
                if training:
                    stdn = stage.tile([128, E], F32, tag="stdn", bufs=2)
                    # softplus(z) = ln(1 + exp(z)) (Softplus LUT not available)
                    nc.scalar.activation(
                        stdn, pg[:, E : 2 * E], mybir.ActivationFunctionType.Exp)
                    nc.vector.tensor_scalar_add(stdn, stdn, 1.0)
                    nc.scalar.activation(
                        stdn, stdn, mybir.ActivationFunctionType.Ln)
                    # (softplus(z)+eps0)*eps
                    nc.vector.scalar_tensor_tensor(
                        stdn, stdn, float(NOISE_EPS), eps_sb[:, bt],
                        op0=mybir.AluOpType.add, op1=mybir.AluOpType.mult)
                    nc.vector.tensor_add(logits, pg[:, 0:E], stdn)
                else:
                    nc.vector.tensor_copy(logits, pg[:, 0:E])
                top8 = stage.tile([128, 8], F32, tag="top8", bufs=2)
                nc.vector.max(top8, logits)
                negmax = stage.tile([128, 1], F32, tag="negmax", bufs=2)
                nc.vector.tensor_scalar(negmax, top8[:, 0:1], -1.0, None,
                                        op0=mybir.AluOpType.mult)
                ex = stage.tile([128, E], F32, tag="ex", bufs=2)
                nc.scalar.activation(ex, logits, mybir.ActivationFunctionType.Exp,
                                     bias=negmax)
                msk = stage.tile([128, E], F32, tag="msk", bufs=2)
                nc.vector.tensor_scalar(msk, logits, top8[:, 2:3], None,
                                        op0=mybir.AluOpType.is_ge)
                nc.vector.tensor_mul(ex, ex, msk)
                ssum = stage.tile([128, 1], F32, tag="ssum", bufs=2)
                nc.vector.reduce_sum(out=ssum, in_=ex, axis=mybir.AxisListType.X)
                rinv = stage.tile([128, 1], F32, tag="rinv", bufs=2)
                nc.vector.reciprocal(rinv, ssum)
                gates = stage.tile([128, E], F32, tag="gates", bufs=2)
                nc.vector.tensor_scalar(gates, ex, rinv, None,
                                        op0=mybir.AluOpType.mult)
                nc.sync.dma_start(out_v[bt], gates)


    nc.compile()
    return nc


_PROGRAM_CACHE = {}


def _get_program(training: bool):
    key = bool(training)
    if key not in _PROGRAM_CACHE:
        _PROGRAM_CACHE[key] = _build_program(key)
    return _PROGRAM_CACHE[key]


def _prep_inputs(x, fc_w, fc_b, w1, b1, w2, b2, w_gate, w_noise, eps):
    """Host-side constant prep shared by all cores."""
    # DFT matrices in f64 then rounded to f32r
    ll = np.arange(L, dtype=np.int64)[:, None]
    ff = np.arange(1, F + 1, dtype=np.int64)[None, :]
    ang = 2.0 * np.pi * ((ll * ff) % L).astype(np.float64) / L
    scale = 1.0 / np.sqrt(L)
    Cm = np.cos(ang) * scale
    Sm = -np.sin(ang) * scale
    csC = Cm.sum(axis=0)
    csS = Sm.sum(axis=0)
    Cr = rnd11(Cm.astype(np.float32))
    Sr = rnd11(Sm.astype(np.float32))

    wgn = np.zeros((F, 512), np.float32)
    wgn[:, 0:E] = w_gate
    wgn[:, E : 2 * E] = w_noise
    wgn = rnd11(wgn)

    bias_all = np.zeros((1, BIAS_LEN), np.float32)
    bias_all[0, OFF_B1R : OFF_B1R + FH] = b1[0]
    bias_all[0, OFF_B1I : OFF_B1I + FH] = b1[1]
    bias_all[0, OFF_B2R : OFF_B2R + F] = b2[0]
    bias_all[0, OFF_B2I : OFF_B2I + F] = b2[1]
    bias_all[0, OFF_CSC : OFF_CSC + F] = csC.astype(np.float32)
    bias_all[0, OFF_CSS : OFF_CSS + F] = csS.astype(np.float32)
    bias_all[0, OFF_ONES : OFF_ONES + 128] = 1.0
    bias_all[0, OFF_FCB : OFF_FCB + 128] = np.float32(fc_b[0])
    bias_all = rnd11(bias_all)

    fcw_rep = np.tile(np.asarray(fc_w, np.float32).reshape(1, CH), (128, 1))
    fcw_rep = np.ascontiguousarray(fcw_rep)

    common = {
        "cdft": Cr,
        "sdft": Sr,
        "w1r": rnd11(w1[0]),
        "w1i": rnd11(w1[1]),
        "w2r": rnd11(w2[0]),
        "w2i": rnd11(w2[1]),
        "wgn": wgn,
        "bias_all": bias_all,
        "fcw": fcw_rep,
    }

    x = np.ascontiguousarray(np.asarray(x, np.float32))
    eps = np.asarray(eps, np.float32)
    in_maps = []
    for i in range(NCORES):
        sh = dict(common)
        sh["x"] = x[i * BL : (i + 1) * BL]
        esh = eps[i * BL : (i + 1) * BL]  # [256, E]
        sh["eps"] = np.ascontiguousarray(
            esh.reshape(2, 128, E).transpose(1, 0, 2))
        in_maps.append(sh)
    return in_maps


def run(inputs, trace=False):
    """Returns (gates [B, E] float32, BassKernelResults)."""
    x = np.asarray(inputs["x"], np.float32)
    fc_w = np.asarray(inputs["fc_w"], np.float32)
    fc_b = np.asarray(inputs["fc_b"], np.float32)
    w1 = np.asarray(inputs["w1"], np.float32)
    b1 = np.asarray(inputs["b1"], np.float32)
    w2 = np.asarray(inputs["w2"], np.float32)
    b2 = np.asarray(inputs["b2"], np.float32)
    w_gate = np.asarray(inputs["w_gate"], np.float32)
    w_noise = np.asarray(inputs["w_noise"], np.float32)
    eps = np.asarray(inputs["eps"], np.float32)
    training = bool(int(np.asarray(inputs.get("training", 1))))

    nc = _get_program(training)
    in_maps = _prep_inputs(x, fc_w, fc_b, w1, b1, w2, b2, w_gate, w_noise, eps)
    res = run_bass_kernel_spmd(
        nc, in_maps, core_ids=list(range(NCORES)), trace=trace,
    )
    gates = np.concatenate([r["out"] for r in res.results], axis=0)
    return gates.astype(np.float32), res


def kernel(**inputs):
    gates, _ = run(inputs, trace=False)
    return gates


# revision 32
# speedup vs baseline: 1.0821x; 1.0821x over previous
"""Trainium2 Bass kernel for nn_AdaptiveFourierTransformGateLayer.

Pipeline (per core, data-parallel over batch across 8 cores):
  x [256,2048,16] --DVE--> h [256,2048] --PE transpose--> hT
  hT --PE DFT matmuls (f32r)--> xr,xi [256,1024] --transpose--> xrT,xiT
  xrT --PE complex MLP layer1 (f32r)--> o1 --transpose--> o1T
  o1T --PE layer2 (split psums A,B,C,D)--> o2r,o2i -> amp --transpose--> ampT
  ampT --PE gate matmul--> logits,z -> noisy-top3 softmax -> gates [256,88]

All heavy matmuls use float32r (11-bit mantissa fp32, full PE rate at
free dim >= 256); weights pre-rounded host-side so DMA feeds f32r tiles
directly. Biases are added via k=1 ones-row matmuls into PSUM.
"""
import sys
import types
import contextlib
import ctypes

import numpy as np

if "/opt/trn_rl_repo" not in sys.path:
    sys.path.insert(0, "/opt/trn_rl_repo")

# ---------------------------------------------------------------------------
# NTFF trace hook shim (only used when trace=True; harmless otherwise)
# ---------------------------------------------------------------------------


def _install_trace_shim():
    if "antenv.axon_hooks" in sys.modules:
        return
    so_path = "/opt/axon/libaxon_pjrt.so"

    def _mk():
        try:
            lib = ctypes.CDLL(so_path)
        except OSError:
            return None
        if not hasattr(lib, "axon_start_nrt_profile"):
            return None
        lib.axon_start_nrt_profile.argtypes = [
            ctypes.POINTER(ctypes.c_int64),
            ctypes.c_size_t,
        ]
        lib.axon_start_nrt_profile.restype = ctypes.c_int64
        lib.axon_stop_nrt_profile.argtypes = [ctypes.c_char_p]
        lib.axon_stop_nrt_profile.restype = ctypes.c_int64

        @contextlib.contextmanager
        def _hook(output_dir, device_ids):
            import jax

            jax.devices()
            if device_ids:
                ids = (ctypes.c_int64 * len(device_ids))(*device_ids)
                rc = lib.axon_start_nrt_profile(ids, len(device_ids))
            else:
                rc = lib.axon_start_nrt_profile(None, 0)
            if rc != 0:
                raise RuntimeError(f"axon_start_nrt_profile rc={rc}")
            try:
                yield
            finally:
                n = lib.axon_stop_nrt_profile(str(output_dir).encode())
                print(f"profile: {n} file(s) written to {output_dir}", file=sys.stderr)

        return _hook

    mod = types.ModuleType("antenv.axon_hooks")
    mod._hook = _mk()
    mod.get_axon_ntff_profile_hook = lambda: mod._hook
    mod.set_axon_ntff_profile_hook = lambda h: setattr(mod, "_hook", h)
    sys.modules["antenv.axon_hooks"] = mod
    try:
        import antenv

        antenv.axon_hooks = mod
    except ImportError:
        pass


_install_trace_shim()

import concourse.tile as tile  # noqa: E402
from concourse import bacc, mybir  # noqa: E402
from concourse.bass_utils import run_bass_kernel_spmd  # noqa: E402
from concourse.masks import make_identity  # noqa: E402

# ---------------------------------------------------------------------------
# Problem constants (hardcoded)
# ---------------------------------------------------------------------------
B = 2048
L = 2048
CH = 16
F = 1024  # num freqs (rfft bins 1..1024)
FH = 4096  # hidden
E = 88  # num experts
TOPK = 3
NOISE_EPS = 0.01
NCORES = 8
BL = B // NCORES  # 256 rows per core
F32R = mybir.dt.float32r
F32 = mybir.dt.float32


def rnd11(x):
    """Round-to-nearest keeping 11 mantissa bits (hardware f32r rounding)."""
    a = np.ascontiguousarray(x, np.float32)
    ai = a.view(np.uint32)
    return ((ai + np.uint32(1 << 11)) & np.uint32(0xFFFFF000)).view(np.float32)


# bias_all layout offsets (elements)
OFF_B1R = 0
OFF_B1I = 4096
OFF_B2R = 8192
OFF_B2I = 9216
OFF_CSC = 10240
OFF_CSS = 11264
OFF_ONES = 12288
OFF_FCB = 12416
BIAS_LEN = 12544


def _build_program(training: bool):
    nc = bacc.Bacc("TRN2", target_bir_lowering=False, debug=False, num_devices=NCORES)

    x_d = nc.dram_tensor("x", [BL, L, CH], F32, kind="ExternalInput").ap()
    c_d = nc.dram_tensor("cdft", [L, F], F32R, kind="ExternalInput").ap()
    s_d = nc.dram_tensor("sdft", [L, F], F32R, kind="ExternalInput").ap()
    w1r_d = nc.dram_tensor("w1r", [F, FH], F32R, kind="ExternalInput").ap()
    w1i_d = nc.dram_tensor("w1i", [F, FH], F32R, kind="ExternalInput").ap()
    w2r_d = nc.dram_tensor("w2r", [FH, F], F32R, kind="ExternalInput").ap()
    w2i_d = nc.dram_tensor("w2i", [FH, F], F32R, kind="ExternalInput").ap()
    wgn_d = nc.dram_tensor("wgn", [F, 512], F32R, kind="ExternalInput").ap()
    bias_d = nc.dram_tensor("bias_all", [1, BIAS_LEN], F32R, kind="ExternalInput").ap()
    fcw_d = nc.dram_tensor("fcw", [128, CH], F32, kind="ExternalInput").ap()
    eps_d = nc.dram_tensor("eps", [128, 2, E], F32, kind="ExternalInput").ap()
    out_d = nc.dram_tensor("out", [BL, E], F32, kind="ExternalOutput").ap()

    with tile.TileContext(nc) as tc:
        with tc.tile_pool(name="acts", bufs=1) as acts, \
             tc.tile_pool(name="stream", bufs=4) as stream, \
             tc.tile_pool(name="stage", bufs=2) as stage, \
             tc.tile_pool(name="stageb", bufs=2) as stageb, \
             tc.tile_pool(name="consts", bufs=1) as consts, \
             tc.tile_pool(name="ps", bufs=2, space="PSUM") as ps:

            ident = consts.tile([128, 128], F32, tag="ident")
            make_identity(nc, ident)
            ident_r = consts.tile([128, 128], F32R, tag="identr")
            nc.vector.tensor_copy(ident_r, ident)
            fcw = consts.tile([128, CH], F32, tag="fcw")
            nc.sync.dma_start(fcw, fcw_d)
            eps_sb = consts.tile([128, 2, E], F32, tag="eps")
            nc.sync.dma_start(eps_sb, eps_d)
            ones_row = consts.tile([1, 128], F32R, tag="ones")
            nc.sync.dma_start(ones_row, bias_d[:, OFF_ONES : OFF_ONES + 128])
            fcb_row = consts.tile([1, 128], F32R, tag="fcb")
            nc.sync.dma_start(fcb_row, bias_d[:, OFF_FCB : OFF_FCB + 128])

            # persistent activation tiles
            hT = acts.tile([128, 16, 256], F32R, tag="hT")  # [l-part, lt, b]
            xrT = acts.tile([128, 8, 256], F32R, tag="xT")  # [f-part, ft, b]
            xiT = acts.tile([128, 8, 256], F32R, tag="xT2")
            xiTn = acts.tile([128, 8, 256], F32R, tag="xT3")
            o1rT = acts.tile([128, 32, 256], F32R, tag="o1rT")  # [h-part, ht, b]
            o1iT = acts.tile([128, 32, 256], F32R, tag="o1iT")

            # ---------------- Stage A: FC + transpose to hT ----------------
            scopeA = nc.named_scope("stageA_fc"); scopeA.__enter__()
            LC = 256  # l-elems per FC chunk
            fcw_t = consts.tile([128, 32, 8], F32, tag="fcwt")
            nc.vector.tensor_copy(
                fcw_t, fcw[:, 0:8].unsqueeze(1).to_broadcast([128, 32, 8]))
            fcw_b3 = fcw_t.unsqueeze(1).to_broadcast([128, LC // 32, 32, 8])
            x_v = x_d.rearrange("(bt p) l c -> bt p (l c)", bt=2)
            cv = c_d.rearrange("(g p) f -> p g f", p=128)
            sv = s_d.rearrange("(g p) f -> p g f", p=128)
            for bt in range(2):
                for lc in range(L // LC):
                    xa = stream.tile([128, LC, CH], F32, tag="stream")
                    nc.sync.dma_start(
                        xa.rearrange("p l c -> p (l c)"),
                        x_v[bt][:, lc * LC * CH : (lc + 1) * LC * CH],
                    )
                    # in-place multiply by fc_w: DVE does channels 0..7,
                    # ScalarE does 8..15 (fc_w replicated across partitions)
                    xa4 = xa.rearrange("p (g q) c -> p g q c", q=32)
                    nc.vector.tensor_tensor(
                        xa4[:, :, :, 0:8], xa4[:, :, :, 0:8], fcw_b3,
                        op=mybir.AluOpType.mult)
                    for c in range(8, 16):
                        nc.scalar.mul(xa[:, :, c], xa[:, :, c], fcw[:, c : c + 1])
                    hch = stage.tile([128, LC], F32, tag="stage")
                    nc.vector.tensor_reduce(
                        out=hch, in_=xa, op=mybir.AluOpType.add,
                        axis=mybir.AxisListType.X,
                    )
                    # transpose 128-blocks into hT
                    for j in range(LC // 128):
                        lt = (lc * LC) // 128 + j
                        pt = ps.tile([128, 128], F32, tag="pb", bufs=4)
                        nc.tensor.transpose(pt, hch[:, j * 128 : (j + 1) * 128], ident)
                        nc.vector.tensor_copy(hT[:, lt, bt * 128 : (bt + 1) * 128], pt)

            scopeA.__exit__(None, None, None)
            scopeB = nc.named_scope("stageB_dft"); scopeB.__enter__()
            for f5 in range(2):
                ps_xr = [None, None]
                ps_xi = [None, None]
                for bt in range(2):
                    ps_xr[bt] = ps.tile([128, 512], F32, tag="pa", bufs=4,
                                        name=f"ps_xr_{f5}_{bt}")
                    ps_xi[bt] = ps.tile([128, 512], F32, tag="pb", bufs=4,
                                        name=f"ps_xi_{f5}_{bt}")
                for ltg in range(2):  # groups of 8 lt
                    c_sb = stream.tile([128, 8, 512], F32R, tag="stream",
                                       name=f"c_sb_{f5}_{ltg}")
                    s_sb = stream.tile([128, 8, 512], F32R, tag="stream",
                                       name=f"s_sb_{f5}_{ltg}")
                    nc.sync.dma_start(
                        c_sb, cv[:, ltg * 8 : (ltg + 1) * 8, f5 * 512 : (f5 + 1) * 512])
                    nc.sync.dma_start(
                        s_sb, sv[:, ltg * 8 : (ltg + 1) * 8, f5 * 512 : (f5 + 1) * 512])
                    for j in range(8):
                        lt = ltg * 8 + j
                        for bt in range(2):
                            first = lt == 0
                            nc.tensor.matmul(
                                ps_xr[bt], hT[:, lt, bt * 128 : (bt + 1) * 128],
                                c_sb[:, j], start=first, stop=False)
                            nc.tensor.matmul(
                                ps_xi[bt], hT[:, lt, bt * 128 : (bt + 1) * 128],
                                s_sb[:, j], start=first, stop=False)
                # fc_b contribution: += fc_b * colsum(C/S)
                csc = stageb.tile([1, 512], F32R, tag="stageb",
                                  name=f"csc_{f5}")
                nc.sync.dma_start(
                    csc, bias_d[:, OFF_CSC + f5 * 512 : OFF_CSC + (f5 + 1) * 512])
                css = stageb.tile([1, 512], F32R, tag="stageb",
                                  name=f"css_{f5}")
                nc.sync.dma_start(
                    css, bias_d[:, OFF_CSS + f5 * 512 : OFF_CSS + (f5 + 1) * 512])
                for bt in range(2):
                    nc.tensor.matmul(ps_xr[bt], fcb_row, csc, start=False, stop=True)
                    nc.tensor.matmul(ps_xi[bt], fcb_row, css, start=False, stop=True)
                # evacuate + transpose into xrT/xiT/xiTn
                for bt in range(2):
                    st_r = stage.tile([128, 512], F32, tag="stage",
                                      name=f"dft_er_{f5}_{bt}")
                    nc.scalar.copy(st_r, ps_xr[bt])
                    st_i = stage.tile([128, 512], F32, tag="stage",
                                      name=f"dft_ei_{f5}_{bt}")
                    nc.scalar.copy(st_i, ps_xi[bt])
                    for j in range(4):
                        ft = f5 * 4 + j
                        pt = ps.tile([128, 128], F32, tag="pb", bufs=4)
                        nc.tensor.transpose(pt, st_r[:, j * 128 : (j + 1) * 128], ident)
                        nc.vector.tensor_copy(
                            xrT[:, ft, bt * 128 : (bt + 1) * 128], pt)
                        pt2 = ps.tile([128, 128], F32, tag="pb", bufs=4)
                        nc.tensor.transpose(pt2, st_i[:, j * 128 : (j + 1) * 128], ident)
                        nc.vector.tensor_copy(
                            xiT[:, ft, bt * 128 : (bt + 1) * 128], pt2)
                        nc.vector.tensor_scalar(
                            xiTn[:, ft, bt * 128 : (bt + 1) * 128], pt2,
                            -1.0, None, op0=mybir.AluOpType.mult)

            scopeB.__exit__(None, None, None)
            scopeC = nc.named_scope("stageC_l1"); scopeC.__enter__()
            # o1r = relu(xr@W1r - xi@W1i + b1r) ; o1i = relu(xi@W1r + xr@W1i + b1i)
            w1r_v = w1r_d.rearrange("(ft p) h -> p ft h", p=128)
            w1i_v = w1i_d.rearrange("(ft p) h -> p ft h", p=128)
            for h5 in range(8):  # h chunks of 512
                w1r_sb = stream.tile([128, 8, 512], F32R, tag="stream")
                nc.sync.dma_start(w1r_sb, w1r_v[:, :, h5 * 512 : (h5 + 1) * 512])
                w1i_sb = stream.tile([128, 8, 512], F32R, tag="stream")
                nc.sync.dma_start(w1i_sb, w1i_v[:, :, h5 * 512 : (h5 + 1) * 512])
                b1r_sl = stageb.tile([1, 512], F32R, tag="stageb")
                nc.sync.dma_start(
                    b1r_sl, bias_d[:, OFF_B1R + h5 * 512 : OFF_B1R + (h5 + 1) * 512])
                b1i_sl = stageb.tile([1, 512], F32R, tag="stageb")
                nc.sync.dma_start(
                    b1i_sl, bias_d[:, OFF_B1I + h5 * 512 : OFF_B1I + (h5 + 1) * 512])
                for bt in range(2):
                    bs = slice(bt * 128, (bt + 1) * 128)
                    p_r = ps.tile([128, 512], F32, tag="pa", bufs=4)
                    p_i = ps.tile([128, 512], F32, tag="pb", bufs=4)
                    for ft in range(8):
                        first = ft == 0
                        # ordered so each stationary (lhsT) is reused
                        nc.tensor.matmul(p_r, xrT[:, ft, bs], w1r_sb[:, ft],
                                         start=first, stop=False)
                        nc.tensor.matmul(p_i, xrT[:, ft, bs], w1i_sb[:, ft],
                                         start=first, stop=False)
                        nc.tensor.matmul(p_i, xiT[:, ft, bs], w1r_sb[:, ft],
                                         start=False, stop=False)
                        nc.tensor.matmul(p_r, xiTn[:, ft, bs], w1i_sb[:, ft],
                                         start=False, stop=False)
                    nc.tensor.matmul(p_r, ones_row, b1r_sl, start=False, stop=True)
                    nc.tensor.matmul(p_i, ones_row, b1i_sl, start=False, stop=True)
                    # relu evac -> f32r staging -> transpose -> o1T
                    st_r = stage.tile([128, 512], F32R, tag="stager", bufs=3)
                    nc.scalar.activation(st_r, p_r, mybir.ActivationFunctionType.Relu)
                    st_i = stage.tile([128, 512], F32R, tag="stager", bufs=3)
                    nc.scalar.activation(st_i, p_i, mybir.ActivationFunctionType.Relu)
                    for j in range(4):
                        ht = h5 * 4 + j
                        pt = ps.tile([128, 128], F32R, tag="pa", bufs=4)
                        nc.tensor.transpose(pt, st_r[:, j * 128 : (j + 1) * 128],
                                            ident_r)
                        nc.vector.tensor_copy(o1rT[:, ht, bs], pt)
                        pt2 = ps.tile([128, 128], F32R, tag="pb", bufs=4)
                        nc.tensor.transpose(pt2, st_i[:, j * 128 : (j + 1) * 128],
                                            ident_r)
                        nc.vector.tensor_copy(o1iT[:, ht, bs], pt2)

            scopeC.__exit__(None, None, None)
            scopeD = nc.named_scope("stageD_l2"); scopeD.__enter__()
            # o2r = (A - B) + b2r ; o2i = (C + D) + b2i
            # A=o1r@W2r B=o1i@W2i C=o1i@W2r D=o1r@W2i ; amp=sqrt(o2r^2+o2i^2)
            ampT = acts.tile([128, 8, 256], F32R, tag="ampT")
            w2r_v = w2r_d.rearrange("(hg p) f -> p hg f", p=128)
            w2i_v = w2i_d.rearrange("(hg p) f -> p hg f", p=128)
            amp_sb = consts.tile([128, 2, 2, 512], F32R, tag="ampsb")  # [p, bt, f5, f]
            for f5 in range(2):
                fs = slice(f5 * 512, (f5 + 1) * 512)
                b2r_sl = stageb.tile([1, 512], F32R, tag="stageb")
                nc.sync.dma_start(
                    b2r_sl, bias_d[:, OFF_B2R + f5 * 512 : OFF_B2R + (f5 + 1) * 512])
                b2i_sl = stageb.tile([1, 512], F32R, tag="stageb")
                nc.sync.dma_start(
                    b2i_sl, bias_d[:, OFF_B2I + f5 * 512 : OFF_B2I + (f5 + 1) * 512])
                pA = [None, None]
                pB = [None, None]
                pC = [None, None]
                pD = [None, None]
                for bt in range(2):
                    pA[bt] = ps.tile([128, 512], F32, tag="pa", bufs=4, name=f"pA_{f5}_{bt}")
                    pB[bt] = ps.tile([128, 512], F32, tag="pa", bufs=4, name=f"pB_{f5}_{bt}")
                    pC[bt] = ps.tile([128, 512], F32, tag="pb", bufs=4, name=f"pC_{f5}_{bt}")
                    pD[bt] = ps.tile([128, 512], F32, tag="pb", bufs=4, name=f"pD_{f5}_{bt}")
                for hg in range(4):  # ht groups of 8; W2 chunks loaded ONCE
                    w2r_sb = stream.tile([128, 8, 512], F32R, tag="stream")
                    nc.sync.dma_start(w2r_sb, w2r_v[:, hg * 8 : (hg + 1) * 8, fs])
                    w2i_sb = stream.tile([128, 8, 512], F32R, tag="stream")
                    nc.sync.dma_start(w2i_sb, w2i_v[:, hg * 8 : (hg + 1) * 8, fs])
                    for j in range(8):
                        ht = hg * 8 + j
                        first = ht == 0
                        for bt in range(2):
                            bs = slice(bt * 128, (bt + 1) * 128)
                            # o1rT stationary reused for A and D; o1iT for C, B
                            nc.tensor.matmul(pA[bt], o1rT[:, ht, bs], w2r_sb[:, j],
                                             start=first, stop=False)
                            nc.tensor.matmul(pD[bt], o1rT[:, ht, bs], w2i_sb[:, j],
                                             start=first, stop=(ht == 31))
                            nc.tensor.matmul(pC[bt], o1iT[:, ht, bs], w2r_sb[:, j],
                                             start=first, stop=False)
                            nc.tensor.matmul(pB[bt], o1iT[:, ht, bs], w2i_sb[:, j],
                                             start=first, stop=(ht == 31))
                for bt in range(2):
                    nc.tensor.matmul(pA[bt], ones_row, b2r_sl, start=False, stop=True)
                    nc.tensor.matmul(pC[bt], ones_row, b2i_sl, start=False, stop=True)
                    # amp = sqrt((A-B)^2 + (C+D)^2) -> amp_sb (transposed later)
                    t_r = stage.tile([128, 512], F32, tag="stage")
                    nc.scalar.copy(t_r, pA[bt])
                    nc.vector.tensor_sub(t_r, t_r, pB[bt])
                    t_i = stage.tile([128, 512], F32, tag="stage")
                    nc.scalar.copy(t_i, pC[bt])
                    nc.vector.tensor_add(t_i, t_i, pD[bt])
                    nc.scalar.square(t_r, t_r)
                    nc.scalar.square(t_i, t_i)
                    nc.vector.tensor_add(t_r, t_r, t_i)
                    nc.scalar.sqrt(amp_sb[:, bt, f5], t_r)
            # deferred transposes amp[b, f] -> ampT[f-part, ft, b]
            for bt in range(2):
                bs = slice(bt * 128, (bt + 1) * 128)
                for f5 in range(2):
                    for j in range(4):
                        ft = f5 * 4 + j
                        pt = ps.tile([128, 128], F32R, tag="pa", bufs=4)
                        nc.tensor.transpose(
                            pt, amp_sb[:, bt, f5, j * 128 : (j + 1) * 128], ident_r)
                        nc.vector.tensor_copy(ampT[:, ft, bs], pt)
            scopeD.__exit__(None, None, None)
            scopeE = nc.named_scope("stageE_gate"); scopeE.__enter__()
            wgn_sb = stream.tile([128, 8, 512], F32R, tag="stream")
            nc.sync.dma_start(wgn_sb, wgn_d.rearrange("(ft p) n -> p ft n", p=128))
            out_v = out_d.rearrange("(bt p) e -> bt p e", bt=2)
            for bt in range(2):
                bs = slice(bt * 128, (bt + 1) * 128)
                pg = ps.tile([128, 512], F32, tag="pa", bufs=4)
                for ft in range(8):
                    nc.tensor.matmul(pg, ampT[:, ft, bs], wgn_sb[:, ft],
                                     start=(ft == 0), stop=(ft == 7))
                logits = stage.tile([128, E], F32, tag="logits", bufs=2)
                if training:
                    stdn = stage.tile([128, E], F32, tag="stdn", bufs=2)
                    # softplus(z) = ln(1 + exp(z)) (Softplus LUT not available)
                    nc.scalar.activation(
                        stdn, pg[:, E : 2 * E], mybir.ActivationFunctionType.Exp)
                    nc.vector.tensor_scalar_add(stdn, stdn, 1.0)
                    nc.scalar.activation(
                        stdn, stdn, mybir.ActivationFunctionType.Ln)
                    # (softplus(z)+eps0)*eps
                    nc.vector.scalar_tensor_tensor(
                        stdn, stdn, float(NOISE_EPS), eps_sb[:, bt],
                        op0=mybir.AluOpType.add, op1=mybir.AluOpType.mult)
                    nc.vector.tensor_add(logits, pg[:, 0:E], stdn)
                else:
                    nc.vector.tensor_copy(logits, pg[:, 0:E])
                top8 = stage.tile([128, 8], F32, tag="top8", bufs=2)
                nc.vector.max(top8, logits)
                negmax = stage.tile([128, 1], F32, tag="negmax", bufs=2)
                nc.vector.tensor_scalar(negmax, top8[:, 0:1], -1.0, None,
                                        op0=mybir.AluOpType.mult)
                ex = stage.tile([128, E], F32, tag="ex", bufs=2)
                nc.scalar.activation(ex, logits, mybir.ActivationFunctionType.Exp,
                                     bias=negmax)
                msk = stage.tile([128, E], F32, tag="msk", bufs=2)
                nc.vector.tensor_scalar(msk, logits, top8[:, 2:3], None,
                                        op0=mybir.AluOpType.is_ge)
                nc.vector.tensor_mul(ex, ex, msk)
                ssum = stage.tile([128, 1], F32, tag="ssum", bufs=2)
                nc.vector.reduce_sum(out=ssum, in_=ex, axis=mybir.AxisListType.X)
                rinv = stage.tile([128, 1], F32, tag="rinv", bufs=2)
                nc.vector.reciprocal(rinv, ssum)
                gates = stage.tile([128, E], F32, tag="gates", bufs=2)
                nc.vector.tensor_scalar(gates, ex, rinv, None,
                                        op0=mybir.AluOpType.mult)
                nc.sync.dma_start(out_v[bt], gates)


    nc.compile()
    return nc


_PROGRAM_CACHE = {}


def _get_program(training: bool):
    key = bool(training)
    if key not in _PROGRAM_CACHE:
        _PROGRAM_CACHE[key] = _build_program(key)
    return _PROGRAM_CACHE[key]


def _prep_inputs(x, fc_w, fc_b, w1, b1, w2, b2, w_gate, w_noise, eps):
    """Host-side constant prep shared by all cores."""
    # DFT matrices in f64 then rounded to f32r
    ll = np.arange(L, dtype=np.int64)[:, None]
    ff = np.arange(1, F + 1, dtype=np.int64)[None, :]
    ang = 2.0 * np.pi * ((ll * ff) % L).astype(np.float64) / L
    scale = 1.0 / np.sqrt(L)
    Cm = np.cos(ang) * scale
    Sm = -np.sin(ang) * scale
    csC = Cm.sum(axis=0)
    csS = Sm.sum(axis=0)
    Cr = rnd11(Cm.astype(np.float32))
    Sr = rnd11(Sm.astype(np.float32))

    wgn = np.zeros((F, 512), np.float32)
    wgn[:, 0:E] = w_gate
    wgn[:, E : 2 * E] = w_noise
    wgn = rnd11(wgn)

    bias_all = np.zeros((1, BIAS_LEN), np.float32)
    bias_all[0, OFF_B1R : OFF_B1R + FH] = b1[0]
    bias_all[0, OFF_B1I : OFF_B1I + FH] = b1[1]
    bias_all[0, OFF_B2R : OFF_B2R + F] = b2[0]
    bias_all[0, OFF_B2I : OFF_B2I + F] = b2[1]
    bias_all[0, OFF_CSC : OFF_CSC + F] = csC.astype(np.float32)
    bias_all[0, OFF_CSS : OFF_CSS + F] = csS.astype(np.float32)
    bias_all[0, OFF_ONES : OFF_ONES + 128] = 1.0
    bias_all[0, OFF_FCB : OFF_FCB + 128] = np.float32(fc_b[0])
    bias_all = rnd11(bias_all)

    fcw_rep = np.tile(np.asarray(fc_w, np.float32).reshape(1, CH), (128, 1))
    fcw_rep = np.ascontiguousarray(fcw_rep)

    common = {
        "cdft": Cr,
        "sdft": Sr,
        "w1r": rnd11(w1[0]),
        "w1i": rnd11(w1[1]),
        "w2r": rnd11(w2[0]),
        "w2i": rnd11(w2[1]),
        "wgn": wgn,
        "bias_all": bias_all,
        "fcw": fcw_rep,
    }

    x = np.ascontiguousarray(np.asarray(x, np.float32))
    eps = np.asarray(eps, np.float32)
    in_maps = []
    for i in range(NCORES):
        sh = dict(common)
        sh["x"] = x[i * BL : (i + 1) * BL]
        esh = eps[i * BL : (i + 1) * BL]  # [256, E]
        sh["eps"] = np.ascontiguousarray(
            esh.reshape(2, 128, E).transpose(1, 0, 2))
        in_maps.append(sh)
    return in_maps


def run(inputs, trace=False):
    """Returns (gates [B, E] float32, BassKernelResults)."""
    x = np.asarray(inputs["x"], np.float32)
    fc_w = np.asarray(inputs["fc_w"], np.float32)
    fc_b = np.asarray(inputs["fc_b"], np.float32)
    w1 = np.asarray(inputs["w1"], np.float32)
    b1 = np.asarray(inputs["b1"], np.float32)
    w2 = np.asarray(inputs["w2"], np.float32)
    b2 = np.asarray(inputs["b2"], np.float32)
    w_gate = np.asarray(inputs["w_gate"], np.float32)
    w_noise = np.asarray(inputs["w_noise"], np.float32)
    eps = np.asarray(inputs["eps"], np.float32)
    training = bool(int(np.asarray(inputs.get("training", 1))))

    nc = _get_program(training)
    in_maps = _prep_inputs(x, fc_w, fc_b, w1, b1, w2, b2, w_gate, w_noise, eps)
    res = run_bass_kernel_spmd(
        nc, in_maps, core_ids=list(range(NCORES)), trace=trace,
    )
    gates = np.concatenate([r["out"] for r in res.results], axis=0)
    return gates.astype(np.float32), res


def kernel(**inputs):
    gates, _ = run(inputs, trace=False)
    return gates
